# revision 13
# baseline (speedup 1.0000x reference)
"""Trainium2 Bass kernel for nn_DNN_24464133718540 (embedding_lookup).

Reference computation:
    emb[b,f]  = tables[f, src[b,f]]            # [B, 45, 256] gather
    h         = emb @ W1 + b1                  # [B, 45, 32]
    out[b,f]  = h @ W2 + b2                    # [B, 45, 1]
    result[b] = sum_f out[b,f]                 # [B, 1]

The MLP is linear (no activation), so with w = W1 @ W2 ([256]) and
c = b1 @ W2 + b2 (scalar):
    result[b] = sum_f tables[f, src[b,f]] . w  +  45 * c

Device kernel (SPMD over 8 cores, features sharded 6/6/6/6/6/5/5/5 with
zero-padding to 6 slots):
  phase 1: stream the core's 6 tables from HBM in ~1 MB chunks; fused
           DVE tensor_tensor_reduce computes per-row dot products with w
           -> scores columns [128 v-partitions, 80 chunk-cols] per table.
  phase 2: PE transpose -> PSUM [79,128]; DMA-flatten to a score row
           [1, 10112]; PE K=1 matmul against ones broadcasts the row to
           all 128 partitions (ScalarE evacuates PSUM -> SBUF).
  phase 3: gpsimd ap_gather: 8 blocks of 16 partitions, each block
           gathers 2048 batch indices from its replicated score row.
  phase 4: DMA one row per block -> DRAM out [6, 8, 2048].
Host: sum the 48 partial rows across cores, add 45*c, reshape [B, 1].
"""

import numpy as np

B, F, V, D, H = 16384, 45, 10000, 256, 32
NF = 6                 # feature slots per core (zero-padded)
NCORES = 8
VCH = 80               # score columns per table (9 full chunks x8 + last x8)
VPAD = VCH * 128       # 10240 flattened score-row length (incl. garbage tail)
NBLK = 8               # batch blocks for the gather
BLK = B // NBLK        # 2048 indices per block

# stream layout: chunk c9<9 covers v in [c9*1024,(c9+1)*1024) as [p=128, j=8]
# with v = c9*1024 + p*8 + j; chunk 9 covers [9216,10000) as [p=98, j=8].
# score(v) lands at cols[p, c9*8+j] -> flattened row position col*128 + p.


def _v_to_pos(v):
    """flattened score-row position for vocab index v (vectorized)."""
    c9 = v // 1024
    r = v % 1024
    return (c9 * 8 + (r % 8)) * 128 + r // 8

_COMPILED = {}


def _feature_slots():
    """feature assignment per core: 6,6,6,6,6,5,5,5."""
    counts = [6, 6, 6, 6, 6, 5, 5, 5]
    slots, start = [], 0
    for c in counts:
        slots.append(list(range(start, start + c)))
        start += c
    assert start == F
    return slots


def _build_program():
    import concourse.bacc as bacc
    import concourse.tile as tile
    from concourse import mybir

    f32 = mybir.dt.float32
    bf16 = mybir.dt.bfloat16
    i16 = mybir.dt.int16

    nc = bacc.Bacc("TRN2", target_bir_lowering=False, debug=False,
                   num_devices=NCORES)

    tables_c = nc.dram_tensor("tables_c", [NF, V, D], f32, kind="ExternalInput")
    w_rep_d = nc.dram_tensor("w_rep", [128, D], f32, kind="ExternalInput")
    ident_d = nc.dram_tensor("ident", [128, 128], f32, kind="ExternalInput")
    idx_d = nc.dram_tensor("idx16", [NF, 128, NBLK * BLK // (16 * NBLK)], i16,
                           kind="ExternalInput")  # [NF, 128, 128]
    out_d = nc.dram_tensor("out_part", [NF, NBLK, BLK], f32, kind="ExternalOutput")

    SROW = BLK // 16  # 128 int16 idx entries per partition per feature

    with tile.TileContext(nc) as tc:
        with (
            tc.tile_pool(name="const", bufs=1) as const_pool,
            tc.tile_pool(name="stream", bufs=6) as stream_pool,
            tc.tile_pool(name="prod", bufs=2) as prod_pool,
            tc.tile_pool(name="cols", bufs=3) as cols_pool,
            tc.tile_pool(name="row", bufs=2) as row_pool,
            tc.tile_pool(name="rep", bufs=2) as rep_pool,
            tc.tile_pool(name="gout", bufs=2) as gout_pool,
            tc.tile_pool(name="pst", bufs=2, space="PSUM") as psum_t_pool,
        ):
            # one-time constants
            w_rep = const_pool.tile([128, D], f32, tag="w")
            nc.sync.dma_start(w_rep[:], w_rep_d.ap())
            ident_t = const_pool.tile([128, 128], f32, tag="ident")
            nc.sync.dma_start(ident_t[:], ident_d.ap())
            idx_t = const_pool.tile([128, NF * SROW], i16, tag="idx")
            nc.sync.dma_start(
                idx_t[:].rearrange("p (f s) -> p f s", f=NF),
                idx_d.ap().rearrange("f p s -> p f s"))

            tab_ap = tables_c.ap()  # [NF, V, D]

            # A dma_start whose dependency semaphore is not yet satisfied
            # stalls the *issuing engine's* instruction queue (the wait sits
            # on the doorbell, not the ring descriptor). So phase 2 is split
            # in two stages pipelined at different depths: by the time each
            # doorbell is reached, its wait is already satisfied and the
            # scalar ring keeps streaming table chunks without stalls.
            rep_hold = {}

            def phase2a(f, cols):
                # transpose -> PSUM evac -> flatten row into rep partition 0.
                # Issued one table late: the transpose's input is complete, so
                # the whole chain runs back-to-back with no engine stalls.
                pt = psum_t_pool.tile([VCH, 128], f32, tag="pt")
                nc.tensor.transpose(pt[:], cols[:, :VCH], ident_t[:])
                ptsb = row_pool.tile([VCH, 128], f32, tag="ptsb")
                # PSUM evac on DVE, not ScalarE: ScalarE must stay a pure
                # doorbell queue or a compute op waiting on the previous
                # table's DVE end gates the next table's stream doorbells
                # behind it (zero prefetch depth for the odd chunks).
                nc.vector.tensor_copy(ptsb[:], pt[:])
                rep = rep_pool.tile([128, VPAD], f32, tag="rep")
                nc.scalar.dma_start(
                    rep[0:1, :].rearrange("o (c p) -> o c p", c=VCH), ptsb[:])
                rep_hold[f] = rep

            def phase2b(f):
                # 7 independent 40KB copies p0 -> p{16,32,...,112} (the gather
                # reads only each 16-partition group's base partition), then
                # gather + output. Issued two tables late: the flatten
                # finished a whole table ago, so no doorbell ever waits.
                rep = rep_hold.pop(f)
                repv = rep[:].rearrange("(a g) n -> a g n", g=16)
                for a in range(1, 8):
                    nc.scalar.dma_start(repv[a:a + 1, 0], repv[0:1, 0])

                gout = gout_pool.tile([128, BLK], f32, tag="gout")
                nc.gpsimd.ap_gather(
                    out_ap=gout[:],
                    in_ap=rep[:, :VPAD],
                    idxs_ap=idx_t[:, f * SROW:(f + 1) * SROW],
                    channels=128,
                    num_elems=VPAD,
                    d=1,
                    num_idxs=BLK,
                )
                nc.gpsimd.dma_start(
                    out_d.ap()[f],
                    gout[:].rearrange("(k g) n -> k g n", g=16)[:, 0, :])

            cols_hold = {}
            for f in range(NF):
                # issue the deferred phase-2 work for earlier tables FIRST so
                # every doorbell's wait is satisfied (or within ~2us) when the
                # issuing engine reaches it, keeping the stream rings flowing.
                if f >= 1:
                    phase2a(f - 1, cols_hold.pop(f - 1))
                if f >= 2:
                    phase2b(f - 2)
                cols = cols_pool.tile([128, VCH], f32, tag="cols")
                nc.vector.memset(cols[:], 0.0)
                # ---- phase 1: stream + mul + tree-reduce ----
                # 9 full chunks of 1024 v ([p=128, j=8]), then [p=98, j=8].
                # Chunks alternate between the two HW-DGE rings (sync/scalar)
                # so table streaming is not capped by one ring's bandwidth.
                for c9 in range(10):
                    p = 128 if c9 < 9 else 98
                    st = stream_pool.tile([128, 8 * D], f32, tag="st")
                    src_ap = tab_ap[f, c9 * 1024:c9 * 1024 + p * 8, :].rearrange(
                        "(p j) d -> p j d", p=p)
                    ring = nc.sync if c9 % 2 == 0 else nc.scalar
                    ring.dma_start(st[:p], src_ap)
                    # fp32 mul runs at DVE 1x (fp32 source); the reduction is
                    # split into two bf16 pairwise adds (2x packed mode) plus a
                    # short 1x tensor_reduce over the last 64 elements.
                    prod = prod_pool.tile([128, 8 * D], bf16, tag="prod")
                    nc.vector.tensor_mul(
                        prod[:p].rearrange("p (j d) -> p j d", j=8),
                        st[:p].rearrange("p (j d) -> p j d", j=8),
                        w_rep[:p].unsqueeze(1).broadcast_to([p, 8, D]),
                    )
                    ph = prod_pool.tile([128, 8 * 128], bf16, tag="ph")
                    pv = prod[:p].rearrange("p (j h d) -> p j h d", j=8, h=2)
                    nc.vector.tensor_add(
                        ph[:p].rearrange("p (j d) -> p j d", j=8),
                        pv[:, :, 0], pv[:, :, 1])
                    pq = prod_pool.tile([128, 8 * 64], bf16, tag="pq")
                    phv = ph[:p].rearrange("p (j h d) -> p j h d", j=8, h=2)
                    nc.vector.tensor_add(
                        pq[:p].rearrange("p (j d) -> p j d", j=8),
                        phv[:, :, 0], phv[:, :, 1])
                    nc.vector.tensor_reduce(
                        cols[:p, c9 * 8:(c9 + 1) * 8],
                        pq[:p].rearrange("p (j d) -> p j d", j=8),
                        axis=mybir.AxisListType.X,
                        op=mybir.AluOpType.add,
                    )

                cols_hold[f] = cols
            phase2a(NF - 1, cols_hold.pop(NF - 1))
            phase2b(NF - 2)
            phase2b(NF - 1)

    nc.compile()
    return nc


def _get_program():
    if "nc" not in _COMPILED:
        _COMPILED["nc"] = _build_program()
    return _COMPILED["nc"]


def kernel(src, tables, W1, b1, W2, b2, _trace=False, _trace_cores=None,
           _tmpdir=None):
    from concourse.bass_utils import run_bass_kernel_spmd

    src = np.asarray(src)
    out_dtype = np.float32
    tables = np.asarray(tables, dtype=np.float32)
    W1 = np.asarray(W1, dtype=np.float32)
    b1 = np.asarray(b1, dtype=np.float32)
    W2 = np.asarray(W2, dtype=np.float32)
    b2 = np.asarray(b2, dtype=np.float32)

    w = (W1 @ W2).reshape(D)                      # [256]
    c = float(b1 @ W2[:, 0] + b2[0])              # scalar per feature
    w_rep = np.ascontiguousarray(np.broadcast_to(w[None, :], (128, D)),
                                 dtype=np.float32)
    ident = np.eye(128, dtype=np.float32)

    slots = _feature_slots()
    src_i = np.asarray(src, dtype=np.int64)

    in_maps = []
    for core in range(NCORES):
        feats = slots[core]
        tc_arr = np.zeros((NF, V, D), dtype=np.float32)
        for i, fg in enumerate(feats):
            tc_arr[i] = tables[fg]
        idx16 = np.zeros((NF, 128, BLK // 16), dtype=np.int16)
        for i, fg in enumerate(feats):
            col = _v_to_pos(src_i[:, fg]).astype(np.int16)   # [16384] row positions
            # idx16[i, 16k+p, s] = pos(src[2048k + 16s + p])
            idx16[i] = (col.reshape(NBLK, BLK // 16, 16)
                        .transpose(0, 2, 1)
                        .reshape(128, BLK // 16))
        in_maps.append({
            "tables_c": tc_arr,
            "w_rep": w_rep,
            "ident": ident,
            "idx16": idx16,
        })

    nc = _get_program()
    kw = {}
    if _trace:
        kw = {"trace": True, "trace_cores": _trace_cores or [0],
              "tmpdir": _tmpdir}
    res = run_bass_kernel_spmd(nc, in_maps, core_ids=list(range(NCORES)), **kw)
    _COMPILED["last_results"] = res

    total = np.zeros(B, dtype=np.float64)
    for core in range(NCORES):
        part = res.results[core]["out_part"].reshape(NF, B)
        nf = len(slots[core])
        total += part[:nf].sum(axis=0, dtype=np.float64)
    total += F * c
    return total.astype(out_dtype).reshape(B, 1)

